# revision 1
# baseline (speedup 1.0000x reference)
"""Causal multi-head attention (b=4, s=2048, d_model=1024, 16 heads) on 8 TRN2
NeuronCores via Bass/Tile.

Sharding: core c = (batch b = c//2, head-group g = c%2). Each core computes its
batch's attention for 8 heads (column-split W_Q/W_K/W_V, row-split W_O) and
returns a partial [2048, 1024] output; the host sums the two head-group
partials per batch.

Device dataflow (all matmuls in float32r = fp32 with 12-bit mantissa, which
streams at bf16 speed on the PE; inputs pre-rounded on the host so no on-device
rounding passes are needed):

  QT/KT:  [d_head, s]-layout projections, 2 heads packed per 128 partitions
  V:      [s, d]-layout projection, augmented with a ones column per head so
          the attention-value matmul also emits softmax row sums (M=65)
  scores: S^T[k, q] tiles via K=64 row-packed matmul pairs (2 heads concurrent)
  P:      exp on ACT straight from 2-bank PSUM, causal-zeroed via affine_select
  z^T:    [V_h | 1].T @ P^T accumulation over k-tiles -> rows 0:64 = z^T,
          row 64 = sums
  1/r:    DVE reciprocal of sums, broadcast across partitions by a K=1 fp32
          ones-matmul, applied to z^T during the PSUM->SBUF copy
  out:    zhat^T.T @ W_O^T -> [s, d_model] partial, DMA'd out in natural layout
"""
import sys

sys.path.insert(0, "/opt/trn_rl_repo")

import numpy as np
from contextlib import ExitStack

import concourse.bass as bass
import concourse.mybir as mybir
import concourse.tile as tile
from concourse import bacc
from concourse import bass_utils

F32 = mybir.dt.float32
F32R = mybir.dt.float32r

S = 2048          # sequence length
D = 1024          # model dim
GH = 8            # heads per core (group)
DH = 64           # head dim
NPAIR = GH // 2   # head pairs per core
NKC = D // 128    # contraction chunks
NSC = S // 512    # s-chunks (proj) == q-chunks (attention)
NST = S // 128    # s-tiles of 128 == k-tiles
N_CORES = 8


def round_f32r(x: np.ndarray) -> np.ndarray:
    """Round-to-nearest-even to float32r (fp32 with low 12 mantissa bits zero)."""
    u = np.ascontiguousarray(x, dtype=np.float32).view(np.uint32)
    r = (u + 0x7FF + ((u >> 12) & 1)) & np.uint32(0xFFFFF000)
    return r.view(np.float32)


def build(reps=1):
    nc = bacc.Bacc("TRN2", target_bir_lowering=False, debug=False)

    xt_d = nc.dram_tensor("xt", [D, S], F32R, kind="ExternalInput").ap()
    wq_d = nc.dram_tensor("wq", [D, GH * DH], F32R, kind="ExternalInput").ap()
    wk_d = nc.dram_tensor("wk", [D, GH * DH], F32R, kind="ExternalInput").ap()
    wv_d = nc.dram_tensor("wv", [D, GH * DH], F32R, kind="ExternalInput").ap()
    wo_d = nc.dram_tensor("wo", [GH * DH, D], F32R, kind="ExternalInput").ap()
    out_d = nc.dram_tensor("out", [S, D], F32, kind="ExternalOutput").ap()

    with tile.TileContext(nc) as tc:
        for _rep in range(reps):
            _emit_body(nc, tc, xt_d, wq_d, wk_d, wv_d, wo_d, out_d)

    nc.compile()
    return nc


def _emit_body(nc, tc, xt_d, wq_d, wk_d, wv_d, wo_d, out_d):
    EXP = mybir.ActivationFunctionType.Exp

    with ExitStack() as ctx:
        wpool = ctx.enter_context(tc.tile_pool(name="w", bufs=1))
        xpool = ctx.enter_context(tc.tile_pool(name="x", bufs=2))  # 2 x 8KB chunks
        qkv = ctx.enter_context(tc.tile_pool(name="qkv", bufs=1))
        ppool = ctx.enter_context(tc.tile_pool(name="p2", bufs=3))
        zpool = ctx.enter_context(tc.tile_pool(name="zhat", bufs=1))
        rpool = ctx.enter_context(tc.tile_pool(name="rr", bufs=1))
        rbpool = ctx.enter_context(tc.tile_pool(name="rbc", bufs=1))
        opool = ctx.enter_context(tc.tile_pool(name="osb", bufs=2))
        # PSUM: sc(2 tiles x 2 banks) + zt(3 x 1) + shared proj/W_O/bcast(1)
        scp = ctx.enter_context(tc.tile_pool(name="scp", bufs=2, space="PSUM"))
        ztp = ctx.enter_context(tc.tile_pool(name="ztp", bufs=3, space="PSUM"))
        pop = ctx.enter_context(tc.tile_pool(name="pop", bufs=1, space="PSUM"))

        # --- weights ---
        wq = wpool.tile([128, NKC, 512], F32R)
        wk = wpool.tile([128, NKC, 512], F32R)
        wv = wpool.tile([128, NKC, 512], F32R)
        wo = wpool.tile([128, NPAIR, D], F32R)
        for kc in range(NKC):
            nc.sync.dma_start(wq[:, kc, :], wq_d[kc * 128:(kc + 1) * 128, :])

        # --- constants ---
        ones128 = wpool.tile([128, 1], F32)
        nc.vector.memset(ones128[:], 1.0)
        ones1_f = wpool.tile([1, 128], F32)
        nc.vector.memset(ones1_f[:], 1.0)
        ones1 = wpool.tile([1, 128], F32R)
        nc.vector.tensor_copy(ones1[:], ones1_f[:])

        # --- persistent activations ---
        qtpool = ctx.enter_context(tc.tile_pool(name="qt", bufs=2))
        kt_t = qkv.tile([128, NPAIR, S], F32R)   # K^T, pair-packed
        vaug = qkv.tile([128, NST, GH * 65], F32R)  # [V_h | 1] per k-tile

        # ones columns of vaug (written once)
        vav = vaug[:].rearrange("p t (h c) -> p t h c", c=65)
        nc.vector.tensor_copy(
            vav[:, :, :, 64:65],
            ones128[:, None, None, :].broadcast_to([128, NST, GH, 1]),
        )

        # --- projections, per s-chunk of 512, emitted as a queue of matmul
        # groups so they can be drip-fed between attention pair-loops (engines
        # run their streams in order; proj groups fill PE's ACT-gated gaps).
        # Q^T slices live in a 2-deep pool: chunk sc's Q is only read by
        # q-chunk sc's scores. ---
        PCH = 512

        def proj_chunk_groups(sc):
            xr = xpool.tile([128, NKC, PCH], F32R)
            for kc in range(NKC):
                nc.sync.dma_start(
                    xr[:, kc, :],
                    xt_d[kc * 128:(kc + 1) * 128, sc * PCH:(sc + 1) * PCH],
                )
            qtile = qtpool.tile([128, NPAIR, PCH], F32R)

            def qk_group(pair, w_t, dst, dsl):
                def emit():
                    ps = pop.tile([128, PCH], F32, tag="po")
                    for kc in range(NKC):
                        nc.tensor.matmul(
                            ps[:], w_t[:, kc, pair * 128:(pair + 1) * 128],
                            xr[:, kc, :], start=(kc == 0), stop=(kc == NKC - 1),
                        )
                    nc.vector.tensor_copy(dst[:, pair, dsl], ps[:])
                return emit

            def v_group(st):
                def emit():
                    ps = pop.tile([128, 512], F32, tag="po")
                    for kc in range(NKC):
                        nc.tensor.matmul(
                            ps[:], xr[:, kc, st * 128:(st + 1) * 128],
                            wv[:, kc, :], start=(kc == 0), stop=(kc == NKC - 1),
                        )
                    tgl = sc * (PCH // 128) + st
                    nc.vector.tensor_copy(
                        vav[:, tgl, :, 0:64],
                        ps[:].rearrange("p (h c) -> p h c", c=64),
                    )
                return emit

            gs = []
            for pair in range(NPAIR):
                gs.append(qk_group(pair, wq, qtile, slice(0, PCH)))
                gs.append(qk_group(pair, wk, kt_t, slice(sc * PCH, (sc + 1) * PCH)))
            for st in range(PCH // 128):
                gs.append(v_group(st))
            return qtile, gs

        def emit_qc(qc, qtile, pending):
            zhat = zpool.tile([128, NPAIR, 512], F32R)
            for pair in range(NPAIR):
                h0, h1 = 2 * pair, 2 * pair + 1
                nk = 4 * (qc + 1)
                zt0 = ztp.tile([65, 512], F32, tag="zt")
                zt1 = ztp.tile([65, 512], F32, tag="zt")
                for kt in range(nk):
                    ks = slice(kt * 128, (kt + 1) * 128)
                    # diagonal tiles: columns q < o are fully masked, so the
                    # score/exp/AV streams all run on [o, 512) only
                    o = max(0, kt * 128 - qc * 512) if kt >= 4 * qc else 0
                    sc2 = scp.tile([128, 1024], F32, tag="sc")
                    nc.tensor.matmul(
                        sc2[:, o:512], kt_t[0:64, pair, ks],
                        qtile[0:64, pair, o:512], start=True, stop=True,
                    )
                    nc.tensor.matmul(
                        sc2[:, 512 + o:1024], kt_t[64:128, pair, ks],
                        qtile[64:128, pair, o:512], start=True, stop=True,
                    )
                    p2 = ppool.tile([128, 1024], F32R)
                    p2h = p2[:].rearrange("p (h q) -> p h q", h=2)
                    s2h = sc2[:].rearrange("p (h q) -> p h q", h=2)
                    nc.scalar.activation(p2h[:, :, o:512], s2h[:, :, o:512], EXP)
                    if kt >= 4 * qc:
                        p2v = p2h[:, :, o:o + 128]
                        nc.gpsimd.affine_select(
                            out=p2v, in_=p2v,
                            compare_op=mybir.AluOpType.is_ge, fill=0.0,
                            base=0, pattern=[[0, 2], [1, 128]], channel_multiplier=-1,
                        )
                    nc.tensor.matmul(
                        zt0[0:65, o:512], vaug[:, kt, h0 * 65:(h0 + 1) * 65],
                        p2[:, o:512], start=(kt == 0), stop=(kt == nk - 1),
                    )
                    nc.tensor.matmul(
                        zt1[0:65, o:512], vaug[:, kt, h1 * 65:(h1 + 1) * 65],
                        p2[:, 512 + o:1024], start=(kt == 0), stop=(kt == nk - 1),
                    )
                # softmax division: recips issue first (DVE overlaps the
                # dripped proj matmuls below), then the K=1 broadcast matmul
                # and the divide folded into the z^T PSUM->SBUF copy
                rrec = rpool.tile([1, 1024], F32R)
                with nc.allow_low_precision(reason="f32r recip feeds f32r matmul"):
                    nc.vector.reciprocal(rrec[:, 0:512], zt0[64:65, :])
                    nc.vector.reciprocal(rrec[:, 512:1024], zt1[64:65, :])
                for half, zt_h in ((0, zt0), (1, zt1)):
                    rb_ps = pop.tile([128, 512], F32, tag="po")
                    nc.tensor.matmul(
                        rb_ps[:], ones1[:], rrec[:, half * 512:(half + 1) * 512],
                        start=True, stop=True,
                    )
                    rbc_sb = rbpool.tile([128, 512], F32, tag="rbs")
                    nc.vector.tensor_copy(rbc_sb[:], rb_ps[:])
                    pr = slice(64 * half, 64 * half + 64)
                    nc.vector.tensor_mul(
                        zhat[pr, pair, :], zt_h[0:64, :], rbc_sb[pr, :]
                    )
                # drip-feed projection groups for the next q-chunk into the
                # PE stream while ACT chews on this q-chunk's exps
                for _ in range(3):
                    if pending:
                        pending.pop(0)()
            # output projection for this q-chunk
            for qt_i in range(4):
                row0 = qc * 512 + qt_i * 128
                for dmh in range(2):
                    po = pop.tile([128, 512], F32, tag="po")
                    for cc in range(NPAIR):
                        nc.tensor.matmul(
                            po[:], zhat[:, cc, qt_i * 128:(qt_i + 1) * 128],
                            wo[:, cc, dmh * 512:(dmh + 1) * 512],
                            start=(cc == 0), stop=(cc == NPAIR - 1),
                        )
                    osb = opool.tile([128, 512], F32)
                    nc.vector.tensor_copy(osb[:], po[:])
                    nc.sync.dma_start(
                        out_d[row0:row0 + 128, dmh * 512:(dmh + 1) * 512], osb[:]
                    )

        # chunk 0 must be complete before q-chunk 0 starts. DMA order: wq
        # (above), then chunk-0 x, then the remaining weights; Q-projection
        # groups run while wk/wv/wo are still landing.
        qtile0, gs = proj_chunk_groups(0)
        for kc in range(NKC):
            for w_t, w_src in ((wk, wk_d), (wv, wv_d)):
                nc.sync.dma_start(w_t[:, kc, :], w_src[kc * 128:(kc + 1) * 128, :])
        for cc in range(NPAIR):
            nc.sync.dma_start(wo[:, cc, :], wo_d[cc * 128:(cc + 1) * 128, :])
        gs = [gs[i] for i in (0, 2, 4, 6, 1, 3, 5, 7, 8, 9, 10, 11)]  # Q first
        for g in gs:
            g()

        qtiles = {0: qtile0}
        pending = []
        for qc in range(NSC):
            if qc < NSC - 1:
                qtiles[qc + 1], gs = proj_chunk_groups(qc + 1)
                pending.extend(gs)
            emit_qc(qc, qtiles.pop(qc), pending)
            for g in pending:  # flush before the next q-chunk needs them
                g()
            pending = []


_NC = {}
LAST_RESULTS = None


def _get_nc(reps=1):
    if reps not in _NC:
        _NC[reps] = build(reps)
    return _NC[reps]


def kernel(x, W_Q, W_K, W_V, W_O, trace=False):
    global LAST_RESULTS
    x = np.asarray(x, dtype=np.float32)
    W_Q = np.asarray(W_Q, dtype=np.float32)
    W_K = np.asarray(W_K, dtype=np.float32)
    W_V = np.asarray(W_V, dtype=np.float32)
    W_O = np.asarray(W_O, dtype=np.float32)

    scale = 0.125  # 1/sqrt(d_head), exact power of two
    xt_r = [round_f32r(x[b].T) for b in range(4)]
    w_r = []
    for g in range(2):
        sl = slice(g * 512, (g + 1) * 512)
        w_r.append({
            "wq": round_f32r(W_Q[sl, :].T * scale),
            "wk": round_f32r(W_K[sl, :].T),
            "wv": round_f32r(W_V[sl, :].T),
            "wo": round_f32r(W_O[:, sl].T),
        })
    in_maps = [{"xt": xt_r[c // 2], **w_r[c % 2]} for c in range(N_CORES)]

    nc = _get_nc()
    res = bass_utils.run_bass_kernel_spmd(
        nc, in_maps, core_ids=list(range(N_CORES)), trace=trace
    )
    LAST_RESULTS = res

    out = np.empty((4, S, D), dtype=np.float32)
    for b in range(4):
        out[b] = res.results[2 * b]["out"] + res.results[2 * b + 1]["out"]
    return out


def _make_runner(nc, in_maps):
    import jax
    from jax.sharding import Mesh, PartitionSpec
    from jax.experimental.shard_map import shard_map
    from concourse import bass2jax

    bass2jax.install_neuronx_cc_hook()
    part_name = nc.partition_id_tensor.name if nc.partition_id_tensor else None
    in_names, out_names, out_avals, zero_outs = [], [], [], []
    for alloc in nc.m.functions[0].allocations:
        if not isinstance(alloc, mybir.MemoryLocationSet):
            continue
        name = alloc.memorylocations[0].name
        if alloc.kind == "ExternalInput":
            if name != part_name:
                in_names.append(name)
        elif alloc.kind == "ExternalOutput":
            out_names.append(name)
            shape = tuple(alloc.tensor_shape)
            dtype = mybir.dt.np(alloc.dtype)
            out_avals.append(jax.core.ShapedArray(shape, dtype))
            zero_outs.append(np.zeros(shape, dtype))
    n_params = len(in_names)
    all_names = tuple(in_names + out_names + ([part_name] if part_name else []))

    def _exec(*args):
        operands = list(args)
        if part_name is not None:
            operands.append(bass2jax.partition_id_tensor())
        return tuple(bass2jax._bass_exec_p.bind(
            *operands, out_avals=tuple(out_avals), in_names=all_names,
            out_names=tuple(out_names), lowering_input_output_aliases=(),
            sim_require_finite=True, sim_require_nnan=True, nc=nc,
        ))

    devices = jax.devices()[:N_CORES]
    mesh = Mesh(np.asarray(devices), ("core",))
    specs = (PartitionSpec("core"),) * (n_params + len(out_names))
    out_specs = (PartitionSpec("core"),) * len(out_names)
    f = jax.jit(shard_map(_exec, mesh=mesh, in_specs=specs,
                          out_specs=out_specs, check_rep=False),
                keep_unused=True)
    sharding = jax.sharding.NamedSharding(mesh, PartitionSpec("core"))
    concat_in = [
        np.concatenate([np.asarray(m[name]) for m in in_maps], axis=0)
        for name in in_names
    ]
    concat_zeros = [
        np.zeros((N_CORES * z.shape[0], *z.shape[1:]), z.dtype) for z in zero_outs
    ]
    dev_in = [jax.device_put(a, sharding) for a in concat_in + concat_zeros]
    return f, dev_in


def _prep_in_maps(x, W_Q, W_K, W_V, W_O):
    scale = 0.125
    in_maps = []
    for c in range(N_CORES):
        b, g = c // 2, c % 2
        sl = slice(g * 512, (g + 1) * 512)
        in_maps.append({
            "xt": round_f32r(np.asarray(x[b]).T.astype(np.float32)),
            "wq": round_f32r(np.asarray(W_Q)[sl, :].T * scale),
            "wk": round_f32r(np.asarray(W_K)[sl, :].T),
            "wv": round_f32r(np.asarray(W_V)[sl, :].T),
            "wo": round_f32r(np.asarray(W_O)[:, sl].T),
        })
    return in_maps


def bench(x, W_Q, W_K, W_V, W_O, n_iters=24, big_reps=6):
    """Estimate per-execution HW time: marginal per-call time of a NEFF with
    the body repeated big_reps times minus the reps=1 NEFF, divided by the
    extra reps. Cancels dispatch + NEFF-start overhead."""
    import time
    import statistics
    import jax

    in_maps = _prep_in_maps(x, W_Q, W_K, W_V, W_O)

    def marginal(nc):
        f, dev_in = _make_runner(nc, in_maps)
        jax.block_until_ready(f(*dev_in))  # compile + warm
        def run_k(k):
            best = float("inf")
            for _ in range(3):
                t0 = time.perf_counter()
                rs = None
                for _i in range(k):
                    rs = f(*dev_in)
                jax.block_until_ready(rs)
                best = min(best, time.perf_counter() - t0)
            return best
        samples = []
        for _ in range(5):
            t1, tk = run_k(1), run_k(n_iters)
            samples.append((tk - t1) / (n_iters - 1))
        return statistics.median(samples)

    m1 = marginal(_get_nc(1))
    mb = marginal(_get_nc(big_reps))
    per_body_ns = (mb - m1) / (big_reps - 1) * 1e9
    return per_body_ns, {"marginal_1": m1, f"marginal_{big_reps}": mb}

